# revision 23
# baseline (speedup 1.0000x reference)
"""SAGAN-style attention block on 8 TRN2 NeuronCores, data-parallel over batch.

Reference computation (per image, x: [64, 64, 512]):
    theta = x @ W_theta                     [4096, 64]
    phi   = maxpool2x2(x @ W_phi)           [1024, 64]
    g     = maxpool2x2(x @ W_g)             [1024, 256]
    beta  = softmax(theta @ phi.T, axis=-1) [4096, 1024]
    o     = (beta @ g) @ W_o                [4096, 512]
    out   = gamma * o + x
Sharding: batch 16 -> 2 images per core. No collectives.

v2 layout strategy (per image):
  - x arrives pre-cast to bf16 from the host, loaded si-blocked per 512-query
    block qb: xin[qb] = [128p, (si=4) x (c=512)] in ONE dma. Tiles stay
    resident and serve the residual add in phase C (no x reload).
  - xT [c, q] built by PE transposes with a BF16 identity (1 cyc/row; the
    transpose cost is keyed on the moving operand = identity dtype).
  - Projections bf16 x bf16 -> f32 PSUM. thetaT/phiT evacuated as f32r
    (exact scores path); g maxpooled straight out of PSUM into bf16.
  - scoresT [k, q] = phiT.T @ thetaT in f32r; exp on ScalarE -> bf16 tiles
    exa [128k, 8x512q]. Scores are software-pipelined 2 kc ahead of attnV
    so the PE never waits on PSUM evacuation.
  - softmax denominator: 3-level DVE tree-add over exa -> E [128,512],
    then ONE ones-matmul -> [2, q] sums; reciprocal on DVE; tiny PE
    transposes give recs [q-part, 1].
  - attnV tmpT[v, q] accumulated over kc; last kc split per vc so the tT
    bf16 casts overlap the other vc's matmul. out = (tmp @ (gamma*W_o));
    residual (o * recs) + x fused on GpSimd (otherwise idle engine),
    written bf16 and DMA'd per 512-query block.
"""
import sys
import numpy as np

sys.path.insert(0, "/opt/trn_rl_repo")

from contextlib import ExitStack

import ml_dtypes

import concourse.bass as bass
import concourse.tile as tile
from concourse import bacc, mybir
from concourse.bass_utils import run_bass_kernel_spmd

N_CORES = 8
IMG = 2            # images per core
H = W = 64
S = H * W          # 4096 queries per image
SK = S // 4        # 1024 keys after 2x2 maxpool
C = 512
D = C // 8         # 64
V = C // 2         # 256
QT = 512           # query tile
NQT = S // QT      # 8
QS = 128           # query subtile (partition dim)
NQS = QT // QS     # 4

F32 = mybir.dt.float32
F32R = mybir.dt.float32r
BF16 = mybir.dt.bfloat16
MAX = mybir.AluOpType.max
ADD = mybir.AluOpType.add
MULT = mybir.AluOpType.mult
EXP = mybir.ActivationFunctionType.Exp

_CACHED_NC = None


def _build():
    nc = bacc.Bacc("TRN2", target_bir_lowering=False, debug=False,
                   num_devices=N_CORES)
    x_d = nc.dram_tensor("x", [IMG, S, C], BF16, kind="ExternalInput").ap()
    wtp_d = nc.dram_tensor("wtp", [128, 4 * 128], BF16, kind="ExternalInput").ap()
    wg_d = nc.dram_tensor("wg", [128, 4 * V], BF16, kind="ExternalInput").ap()
    wo_d = nc.dram_tensor("wo", [128, 2 * C], BF16, kind="ExternalInput").ap()
    id_d = nc.dram_tensor("ident", [128, 130], BF16, kind="ExternalInput").ap()
    out_d = nc.dram_tensor("out", [IMG, S, C], BF16, kind="ExternalOutput").ap()

    with tile.TileContext(nc) as tc, ExitStack() as ctx:
        # SBUF pools
        const_p = ctx.enter_context(tc.tile_pool(name="const", bufs=1))
        xin_p = ctx.enter_context(tc.tile_pool(name="xin", bufs=IMG * NQT))
        xt_p = ctx.enter_context(tc.tile_pool(name="xt", bufs=1))
        proj_p = ctx.enter_context(tc.tile_pool(name="proj", bufs=1))
        scr_p = ctx.enter_context(tc.tile_pool(name="scr", bufs=2))
        exa_p = ctx.enter_context(tc.tile_pool(name="exa", bufs=2))
        tre_p = ctx.enter_context(tc.tile_pool(name="tre", bufs=2))
        tt_p = ctx.enter_context(tc.tile_pool(name="tt", bufs=2))
        ot_p = ctx.enter_context(tc.tile_pool(name="ot", bufs=2))
        # PSUM pools (8 banks total):
        #   tp x2 (bf16 transposes), sc x2 (scores + summ + recs),
        #   tv0/tv1 x1 (attnV accumulators), mm x2 (proj + Wo)
        psT = ctx.enter_context(tc.tile_pool(name="psT", bufs=2, space="PSUM"))
        psA = ctx.enter_context(tc.tile_pool(name="psA", bufs=2, space="PSUM"))
        psV = ctx.enter_context(tc.tile_pool(name="psV", bufs=1, space="PSUM"))
        psB = ctx.enter_context(tc.tile_pool(name="psB", bufs=2, space="PSUM"))

        # --- constants / weights (all bf16, preformatted on host) ---
        ident_w = const_p.tile([128, 130], BF16, tag="ident", name="ident_w")
        nc.sync.dma_start(ident_w[:], id_d[:])
        ident = ident_w[:, 0:128]
        ones2 = ident_w[:, 128:130]
        wtp = const_p.tile([128, 4 * 128], BF16, tag="wtp", name="wtp")
        wg = const_p.tile([128, 4 * V], BF16, tag="wg", name="wg")
        wo = const_p.tile([128, 2 * C], BF16, tag="wo", name="wo")
        nc.sync.dma_start(wtp[:], wtp_d[:])
        nc.sync.dma_start(wg[:], wg_d[:])
        nc.sync.dma_start(wo[:], wo_d[:])

        xin = [[None] * NQT for _ in range(IMG)]

        for img in range(IMG):
            # ---------- Phase A ----------
            # img0: xT built by PE transposes from the fast-loading xin
            # tiles, interleaved per-qb with the projections so the PE is
            # busy (and HAM-warm) from ~5us with no DMA-transpose waits.
            # img1: xT via XBAR DMA transpose (~53us of descriptor time),
            # fully hidden under img0's attention phase.
            xT = [xt_p.tile([128, S], BF16, tag=f"xT{cc}", name=f"xT{cc}")
                  for cc in range(4)]
            for qb in range(NQT):
                xin[img][qb] = xin_p.tile([128, 4 * C], BF16, tag="xin",
                                          name=f"xin{img}_{qb}")
                src = x_d[img, qb * QT:(qb + 1) * QT, :].rearrange(
                    "(si p) c -> p si c", p=128)
                dst = xin[img][qb].rearrange("p (si c) -> p si c", si=4)
                nc.scalar.dma_start(dst, src)
            if img == 1:
                for cc in range(4):
                    nc.sync.dma_start_transpose(
                        xT[cc][:], x_d[img, :, cc * 128:(cc + 1) * 128])

            def transpose_qb(qb):
                for cc in range(4):
                    tp = psT.tile([128, QT], BF16, tag="tp", name="tp")
                    for si in range(4):
                        nc.tensor.transpose(
                            tp[:, si * 128:(si + 1) * 128],
                            xin[img][qb][:, si * C + cc * 128:
                                         si * C + (cc + 1) * 128],
                            ident[:])
                    nc.vector.tensor_copy(xT[cc][:, qb * QT:(qb + 1) * QT],
                                          tp[:])

            # ---------- Phase B: projections ----------
            thetaT = proj_p.tile([64, S], F32R, tag="thetaT", name="thetaT")
            phiT = proj_p.tile([64, SK], F32R, tag="phiT", name="phiT")
            gTp = [proj_p.tile([128, SK], BF16, tag=f"gTp{vc}", name=f"gTp{vc}")
                   for vc in range(2)]

            def proj_qt(qt):
                qsl = slice(qt * QT, (qt + 1) * QT)
                tp_ps = psB.tile([128, QT], F32, tag="mm", name="mm")
                for cc in range(4):
                    nc.tensor.matmul(tp_ps[:], wtp[:, cc * 128:(cc + 1) * 128],
                                     xT[cc][:, qsl], start=(cc == 0),
                                     stop=(cc == 3))
                nc.scalar.copy(thetaT[:, qsl], tp_ps[0:64, :])
                # phi maxpool 2x2: tensor_reduce over the w-pair straight
                # from PSUM (single PSUM read), then h-pair max in SBUF.
                # PSUM free dim covers 8 rows of w=64: [h2=4, hp=2, w2=32, wp=2]
                m1 = scr_p.tile([64, 256], F32, tag="m1", name="m1")
                pv = tp_ps[64:128, :].rearrange("p (a b c d) -> p a b c d",
                                                b=2, c=32, d=2)
                m1v = m1.rearrange("p (a b c) -> p a b c", a=4, b=2)
                po = phiT[:, qt * 128:(qt + 1) * 128].rearrange(
                    "p (a c) -> p a c", a=4)
                nc.vector.tensor_reduce(m1v, pv, mybir.AxisListType.X, MAX)
                nc.vector.tensor_tensor(po, m1v[:, :, 0, :], m1v[:, :, 1, :], MAX)

                for vc in range(2):
                    g_ps = psB.tile([128, QT], F32, tag="mm", name="mm")
                    for cc in range(4):
                        nc.tensor.matmul(
                            g_ps[:],
                            wg[:, cc * V + vc * 128: cc * V + (vc + 1) * 128],
                            xT[cc][:, qsl], start=(cc == 0), stop=(cc == 3))
                    m2 = scr_p.tile([128, 256], BF16, tag="m2", name="m2")
                    gv = g_ps.rearrange("p (a b c d) -> p a b c d", b=2, c=32, d=2)
                    m2v = m2.rearrange("p (a b c) -> p a b c", a=4, b=2)
                    go = gTp[vc][:, qt * 128:(qt + 1) * 128].rearrange(
                        "p (a c) -> p a c", a=4)
                    nc.vector.tensor_reduce(m2v, gv, mybir.AxisListType.X, MAX)
                    nc.vector.tensor_tensor(go, m2v[:, :, 0, :], m2v[:, :, 1, :], MAX)

            for qt in range(NQT):
                if img == 0:
                    transpose_qb(qt)
                proj_qt(qt)

            # g -> key-major bf16 via PE transposes, 2 kc per [128, 512] tile
            g_aug = []
            for j in range(4):
                tp = psT.tile([128, QT], BF16, tag="tp", name="tp")
                for sub in range(2):
                    kc = 2 * j + sub
                    for vc in range(2):
                        nc.tensor.transpose(
                            tp[:, sub * 256 + vc * 128:sub * 256 + (vc + 1) * 128],
                            gTp[vc][:, kc * 128:(kc + 1) * 128], ident[:])
                ga_t = proj_p.tile([128, QT], BF16, tag=f"gaug{j}", name=f"gaug{j}")
                nc.vector.tensor_copy(ga_t[:], tp[:])
                g_aug.append(ga_t)

            def gsl(kc, vc):
                return g_aug[kc // 2][:, (kc % 2) * 256 + vc * 128:
                                      (kc % 2) * 256 + (vc + 1) * 128]

            # ---------- Phase C: attention, pipelined across q-tiles ----
            # Scores (and their exps) for the next q-tile are issued during
            # the current tile's attnV stream so ScalarE's exp pipeline stays
            # ahead of the PE. Softmax sums come from four 1-column ones-
            # matmuls on the tree-reduced te tile: [q-part, 1] in PSUM
            # directly, no transposes or reductions.
            ones1 = ident_w[:, 128:129]
            exa_t = [None] * (NQT + 1)

            def get_exa(sqt):
                if exa_t[sqt] is None:
                    exa_t[sqt] = exa_p.tile([128, 8 * QT], BF16, tag="exa",
                                            name="exa")
                return exa_t[sqt]

            def score(sqt, kc):
                if sqt >= NQT:
                    return
                e = get_exa(sqt)
                sp = psA.tile([128, QT], F32, tag="sc", name="sc")
                nc.tensor.matmul(sp[:], phiT[:, kc * 128:(kc + 1) * 128],
                                 thetaT[:, sqt * QT:(sqt + 1) * QT],
                                 start=True, stop=True)
                nc.scalar.activation(e[:, kc * QT:(kc + 1) * QT], sp[:], EXP)

            for qt in range(NQT):
                qsl = slice(qt * QT, (qt + 1) * QT)
                exa = get_exa(qt)
                tv_ps = [psV.tile([128, QT], F32, tag=f"tv{vc}", name=f"tv{vc}")
                         for vc in range(2)]
                lv = tre_p.tile([128, 2048], BF16, tag="lv", name="lv")
                mv = tre_p.tile([128, 1024], BF16, tag="mv", name="mv")
                te = tre_p.tile([128, QT], BF16, tag="te", name="te")

                def leaf(j):  # lv[j] = exp(2j) + exp(2j+1), emitted as exps land
                    nc.vector.tensor_tensor(lv[:, j * QT:(j + 1) * QT],
                                            exa[:, 2 * j * QT:(2 * j + 1) * QT],
                                            exa[:, (2 * j + 1) * QT:(2 * j + 2) * QT],
                                            ADD)

                if qt == 0:
                    score(0, 0)
                    score(0, 1)
                for kc in range(7):
                    nc.tensor.matmul(tv_ps[0][:], gsl(kc, 0),
                                     exa[:, kc * QT:(kc + 1) * QT],
                                     start=(kc == 0), stop=False)
                    nc.tensor.matmul(tv_ps[1][:], gsl(kc, 1),
                                     exa[:, kc * QT:(kc + 1) * QT],
                                     start=(kc == 0), stop=False)
                    if kc <= 5:
                        score(qt, kc + 2)
                    else:
                        score(qt + 1, 0)
                    if kc in (1, 3, 5):
                        leaf(kc // 2)
                        if kc == 3:
                            nc.vector.tensor_tensor(mv[:, 0:QT], lv[:, 0:QT],
                                                    lv[:, QT:2 * QT], ADD)
                # last kc split per vc so the tT casts overlap the PE
                tT = [tt_p.tile([128, QT], BF16, tag=f"tt{vc}", name=f"tt{vc}")
                      for vc in range(2)]
                nc.tensor.matmul(tv_ps[0][:], gsl(7, 0), exa[:, 7 * QT:8 * QT],
                                 start=False, stop=True)
                leaf(3)
                nc.vector.tensor_tensor(mv[:, QT:2 * QT], lv[:, 2 * QT:3 * QT],
                                        lv[:, 3 * QT:4 * QT], ADD)
                nc.vector.tensor_tensor(te[:], mv[:, 0:QT], mv[:, QT:2 * QT], ADD)
                nc.vector.tensor_copy(tT[0][:], tv_ps[0][:])
                nc.tensor.matmul(tv_ps[1][:], gsl(7, 1), exa[:, 7 * QT:8 * QT],
                                 start=False, stop=True)
                nc.vector.tensor_copy(tT[1][:], tv_ps[1][:])
                score(qt + 1, 1)

                def wo_mm(qs, pool, tag):
                    o_ps = pool.tile([128, C], F32, tag=tag, name=tag)
                    ssl = slice(qs * 128, (qs + 1) * 128)
                    for vc in range(2):
                        nc.tensor.matmul(o_ps[:], tT[vc][:, ssl],
                                         wo[:, vc * C:(vc + 1) * C],
                                         start=(vc == 0), stop=(vc == 1))
                    return o_ps

                ot = ot_p.tile([128, 4 * C], BF16, tag="ot", name="ot")

                def evac(qs, o_ps):
                    # out = (o * (1/sum)) + x (gamma folded in W_o); GpSimd
                    # can't touch PSUM, so ScalarE scale + GpSimd add for two
                    # subtiles and DVE fused STT for the other two.
                    if qs < 2:
                        os = scr_p.tile([128, C], BF16, tag="os", name="os")
                        nc.scalar.activation(
                            os[:], o_ps[:], mybir.ActivationFunctionType.Copy,
                            scale=rc[:, qs:qs + 1])
                        nc.gpsimd.tensor_tensor(
                            ot[:, qs * C:(qs + 1) * C], os[:],
                            xin[img][qt][:, qs * C:(qs + 1) * C], ADD)
                    else:
                        nc.vector.scalar_tensor_tensor(
                            ot[:, qs * C:(qs + 1) * C], o_ps[:],
                            rc[:, qs:qs + 1],
                            xin[img][qt][:, qs * C:(qs + 1) * C],
                            MULT, ADD)

                o0 = wo_mm(0, psB, "mm")
                rc_ps = psA.tile([128, QT], F32, tag="sc", name="sc")
                for qs in range(NQS):
                    nc.tensor.matmul(rc_ps[:, qs:qs + 1],
                                     te[:, qs * 128:(qs + 1) * 128], ones1,
                                     start=True, stop=True)
                rc = scr_p.tile([128, NQS], F32, tag="rc", name="rc")
                nc.vector.reciprocal(rc[:], rc_ps[:, 0:NQS])
                evac(0, o0)
                o1 = wo_mm(1, psV, "tv0")
                evac(1, o1)
                o2 = wo_mm(2, psB, "mm")
                evac(2, o2)
                o3 = wo_mm(3, psV, "tv1")
                evac(3, o3)
                dst = out_d[img, qt * QT:(qt + 1) * QT, :].rearrange(
                    "(si p) c -> p si c", p=128)
                nc.sync.dma_start(dst, ot.rearrange("p (si c) -> p si c", si=4))

    nc.compile()
    return nc


def _get_nc():
    global _CACHED_NC
    if _CACHED_NC is None:
        _CACHED_NC = _build()
    return _CACHED_NC


def _run(inputs, trace=False, trace_kwargs=None):
    x = np.asarray(inputs["x"], dtype=np.float32)
    wt = np.asarray(inputs["W_theta"], dtype=np.float32)
    wp = np.asarray(inputs["W_phi"], dtype=np.float32)
    wg = np.asarray(inputs["W_g"], dtype=np.float32)
    wo = np.asarray(inputs["W_o"], dtype=np.float32)
    gamma = float(np.asarray(inputs["gamma"], dtype=np.float32))

    BF = ml_dtypes.bfloat16
    x_b = np.ascontiguousarray(x.reshape(N_CORES * IMG, S, C).astype(BF))
    # wtp: per cc chunk [c=128 rows] -> cols [theta(64) | phi(64)]
    wtp = np.zeros((128, 4 * 128), dtype=np.float32)
    wgp = np.zeros((128, 4 * V), dtype=np.float32)
    wop = np.zeros((128, 2 * C), dtype=np.float32)
    for cc in range(4):
        rows = slice(cc * 128, (cc + 1) * 128)
        wtp[:, cc * 128:cc * 128 + D] = wt[rows, :]
        wtp[:, cc * 128 + D:(cc + 1) * 128] = wp[rows, :]
        wgp[:, cc * V:(cc + 1) * V] = wg[rows, :]
    for vc in range(2):
        wop[:, vc * C:(vc + 1) * C] = gamma * wo[vc * 128:(vc + 1) * 128, :]
    ident = np.concatenate([np.eye(128, dtype=np.float32),
                            np.ones((128, 2), dtype=np.float32)], axis=1)

    in_maps = []
    for i in range(N_CORES):
        in_maps.append({
            "x": np.ascontiguousarray(x_b[i * IMG:(i + 1) * IMG]),
            "wtp": wtp.astype(BF), "wg": wgp.astype(BF),
            "wo": wop.astype(BF), "ident": ident.astype(BF),
        })
    nc = _get_nc()
    kw = {}
    if trace:
        kw["trace"] = True
        if trace_kwargs:
            kw["trace_kwargs"] = trace_kwargs
    res = run_bass_kernel_spmd(nc, in_maps, core_ids=list(range(N_CORES)), **kw)
    outs = [np.asarray(res.results[i]["out"]).astype(np.float32).reshape(
        IMG, H, W, C) for i in range(N_CORES)]
    full = np.concatenate(outs, axis=0)
    return full, res


def kernel(**inputs):
    full, _ = _run(inputs, trace=False)
    return full
